# revision 1
# baseline (speedup 1.0000x reference)
"""Trainium2 Bass kernel for nn_CPAMDec_Mix (dual cross-attention, CPAM decoder).

Math (per batch element n):
    q_i = (wq_i @ x_i + bq_i)            # (D, HW)   1x1 conv query
    k_i = y_i @ wk_i.T + bk_i            # (K, D)    linear key
    v_i = y_i @ wv_i.T + bv_i            # (K, C)    linear value
    e   = | q_1.T k_1.T - q_2.T k_2.T |  # (HW, K)
    a   = softmax_K(e)
    out_i = scale * (v_i.T @ a.T) + x_i  # (C, HW)

Sharding: pure data parallel, one batch element per NeuronCore (N=8, 8 cores).
All weights replicated.  Host-side marshaling pre-transposes the small weight
matrices / y tensors so the contraction dim (C) lands on SBUF partitions.

On-chip layout per core (everything streamed over pixel tiles of L=512):
    E^T (K x L) layout keeps softmax results directly usable as the moving
    operand of the output matmul (contract over K).  Softmax over K (the
    partition dim) is done with ones-matmuls: S = 1.T @ exp(E), then
    R = 1/S broadcast back over K partitions with another ones-matmul.
    exp() needs no max-subtraction: energies are |.| >= 0 and bounded
    (~20 for this operator scale), far from fp32 overflow.
    Matmuls run as float32r (fp32 bits, replicated fast path: 1 PE
    cycle/row for moving >= 256 instead of 4 for plain fp32).  The BIR
    verifier requires every f32r matmul operand to be produced as f32r,
    so matmul-feeding DRAM tensors/tiles are declared f32r end-to-end;
    the residual add reads the x tiles bitcast back to f32 (exact bits).
"""

import numpy as np

N, C, H, W, K = 8, 512, 64, 64, 64
HW = H * W          # 4096
D = C // 4          # 128
L = 512             # pixel tile size
NT = HW // L        # 8 tiles
NCH = C // 128      # 4 contraction chunks
P = 128

_CACHE = {}


def _build():
    from contextlib import ExitStack

    import concourse.tile as tile
    from concourse import bacc, mybir

    f32 = mybir.dt.float32
    f32r = mybir.dt.float32r
    bf16 = mybir.dt.bfloat16
    AF = mybir.ActivationFunctionType
    ALU = mybir.AluOpType

    nc = bacc.Bacc("TRN2", target_bir_lowering=False, debug=False)

    def din(name, shape, dt=f32):
        return nc.dram_tensor(name, shape, dt, kind="ExternalInput").ap()

    def dout(name, shape):
        return nc.dram_tensor(name, shape, f32, kind="ExternalOutput").ap()

    x1 = din("x1", [C, HW], f32r)
    x2 = din("x2", [C, HW], f32r)
    # k/v-side tensors come in as bf16 (they feed the bf16 E/U path)
    y1t = din("y1t", [C, K], bf16)
    y2t = din("y2t", [C, K], bf16)
    wq1t = din("wq1t", [C, D], f32r)
    wq2t = din("wq2t", [C, D], f32r)
    wk1t = din("wk1t", [C, D], bf16)
    wk2t = din("wk2t", [C, D], bf16)
    wv1t = din("wv1t", [C, C], bf16)
    wv2t = din("wv2t", [C, C], bf16)
    bq1 = din("bq1", [D, 1])
    bq2 = din("bq2", [D, 1])
    bk1 = din("bk1", [D, 1])
    bk2 = din("bk2", [D, 1])
    bv1 = din("bv1", [1, C], bf16)
    bv2 = din("bv2", [1, C], bf16)
    ones_r = din("ones_r", [1, K], bf16)
    ones_c = din("ones_c", [K, 1], bf16)
    scol = din("scol", [P, 1])  # scale broadcast to 128 partitions (host)
    o1 = dout("o1", [C, HW])
    o2 = dout("o2", [C, HW])

    # chunked (partition-major) views of the DRAM tensors
    x1r = x1.rearrange("(c p) l -> c p l", p=P)
    x2r = x2.rearrange("(c p) l -> c p l", p=P)
    o1r = o1.rearrange("(c p) l -> c p l", p=P)
    o2r = o2.rearrange("(c p) l -> c p l", p=P)
    y1r = y1t.rearrange("(c p) k -> c p k", p=P)
    y2r = y2t.rearrange("(c p) k -> c p k", p=P)
    wq1r = wq1t.rearrange("(c p) d -> c p d", p=P)
    wq2r = wq2t.rearrange("(c p) d -> c p d", p=P)
    wk1r = wk1t.rearrange("(c p) d -> c p d", p=P)
    wk2r = wk2t.rearrange("(c p) d -> c p d", p=P)
    wv1r = wv1t.rearrange("(c p) e -> c p e", p=P)
    wv2r = wv2t.rearrange("(c p) e -> c p e", p=P)

    with tile.TileContext(nc) as tc, ExitStack() as ctx:
        cpool = ctx.enter_context(tc.tile_pool(name="const", bufs=1))

        # --- load replicated constants -------------------------------------
        def load_chunks(name, src_r, nchunks, width, dt=f32r, eng=None):
            t = cpool.tile([P, nchunks * width], dt, name=name, tag=name)
            for j in range(nchunks):
                (eng or nc.sync).dma_start(
                    t[:, j * width:(j + 1) * width], src_r[j])
            return t

        # small k/q-side weights on the load (SP) ring; the big wv tensors
        # ride the otherwise-idle Activation ring so tile-0 x loads aren't
        # queued behind them
        y1s = load_chunks("y1s", y1r, NCH, K, bf16)
        y2s = load_chunks("y2s", y2r, NCH, K, bf16)
        wk1s = load_chunks("wk1s", wk1r, NCH, D, bf16)
        wk2s = load_chunks("wk2s", wk2r, NCH, D, bf16)
        wq1s = load_chunks("wq1s", wq1r, NCH, D)
        wq2s = load_chunks("wq2s", wq2r, NCH, D)
        wv1s = load_chunks("wv1s", wv1r, NCH, C, bf16, eng=nc.scalar)
        wv2s = load_chunks("wv2s", wv2r, NCH, C, bf16, eng=nc.scalar)

        def load1(name, src, shape, dt=f32):
            t = cpool.tile(shape, dt, name=name, tag=name)
            nc.sync.dma_start(t[:], src[:])
            return t

        bq1s = load1("bq1s", bq1, [D, 1])
        bq2s = load1("bq2s", bq2, [D, 1])
        bk1s = load1("bk1s", bk1, [D, 1])
        bk2s = load1("bk2s", bk2, [D, 1])
        bv1s = load1("bv1s", bv1, [1, C], bf16)
        bv2s = load1("bv2s", bv2, [1, C], bf16)
        onrs = load1("onrs", ones_r, [1, K], bf16)
        oncs = load1("oncs", ones_c, [K, 1], bf16)
        scols = load1("scols", scol, [P, 1])

        bk2n = cpool.tile([D, 1], f32, name="bk2n", tag="bk2n")
        nc.scalar.mul(bk2n[:], bk2s[:], -1.0)

        # --- setup: K1t (D,K), K2tn = -(K2t+bk2), V1 (K,C), V2 (K,C) -------
        # bf16: these feed the E/U matmuls (1 cyc/row vs 2 for f32r)
        k1s = cpool.tile([D, K], bf16, name="k1s", tag="k1s")
        k2ns = cpool.tile([D, K], bf16, name="k2ns", tag="k2ns")
        v1s = cpool.tile([K, C], bf16, name="v1s", tag="v1s")
        v2s = cpool.tile([K, C], bf16, name="v2s", tag="v2s")

        with ExitStack() as sctx:
            spsum = sctx.enter_context(
                tc.tile_pool(name="spsum", bufs=1, space="PSUM"))

            for (wks, ys, ks, bias, sc) in (
                    (wk1s, y1s, k1s, bk1s, 1.0),
                    (wk2s, y2s, k2ns, bk2n, -1.0)):
                kp = spsum.tile([D, K], f32, name="kp", tag="kp")
                for j in range(NCH):
                    nc.tensor.matmul(
                        kp[:],
                        wks[:, j * D:(j + 1) * D],
                        ys[:, j * K:(j + 1) * K],
                        start=(j == 0), stop=(j == NCH - 1))
                # ks = sc*kp + bias  (sc=-1, bias=-bk2 negates K2t + bk2)
                nc.scalar.activation(ks[:], kp[:], AF.Identity,
                                     bias=bias[:], scale=sc)

            for (ys, wvs, bvs, vs) in (
                    (y1s, wv1s, bv1s, v1s), (y2s, wv2s, bv2s, v2s)):
                vp = spsum.tile([K, C], f32, name="vp", tag="vp")
                for j in range(NCH):
                    nc.tensor.matmul(
                        vp[:],
                        ys[:, j * K:(j + 1) * K],
                        wvs[:, j * C:(j + 1) * C],
                        start=(j == 0), stop=False)
                # += ones.T @ bv  (broadcast bias add over K partitions)
                nc.tensor.matmul(vp[:], onrs[:], bvs[:], start=False,
                                 stop=True)
                nc.scalar.copy(vs[:], vp[:])

        # --- streaming pools ----------------------------------------------
        xpool = ctx.enter_context(tc.tile_pool(name="xpool", bufs=4))
        qsb = ctx.enter_context(tc.tile_pool(name="qsb", bufs=3))
        softp = ctx.enter_context(tc.tile_pool(name="softp", bufs=3))
        opool = ctx.enter_context(tc.tile_pool(name="opool", bufs=3))
        qpp = ctx.enter_context(tc.tile_pool(name="qpp", bufs=1, space="PSUM"))
        epp = ctx.enter_context(tc.tile_pool(name="epp", bufs=2, space="PSUM"))
        spp = ctx.enter_context(tc.tile_pool(name="spp", bufs=1, space="PSUM"))
        upp = ctx.enter_context(tc.tile_pool(name="upp", bufs=2, space="PSUM"))

        for t in range(NT):
            l0 = t * L
            xts = {}
            for s, xr in ((0, x1r), (1, x2r)):
                # per-stream tile holding all 4 channel chunks side by side.
                # All loads go on the SP HWDGE ring, all stores on the
                # Activation ring: a ring is FIFO, so mixing loads behind
                # compute-gated stores head-of-line-blocks the loads.
                xt = xpool.tile([P, NCH * L], f32r, name=f"x{s}", tag=f"x{s}")
                for j in range(NCH):
                    nc.sync.dma_start(xt[:, j * L:(j + 1) * L],
                                      xr[j][:, l0:l0 + L])
                xts[s] = xt

            qs = []
            for s, (wqs, bqs) in enumerate(((wq1s, bq1s), (wq2s, bq2s))):
                qp = qpp.tile([D, L], f32, name=f"q{s}p", tag=f"q{s}p")
                for j in range(NCH):
                    nc.tensor.matmul(
                        qp[:],
                        wqs[:, j * D:(j + 1) * D],
                        xts[s][:, j * L:(j + 1) * L],
                        start=(j == 0), stop=(j == NCH - 1))
                q = qsb.tile([D, L], bf16, name=f"q{s}s", tag=f"q{s}s")
                nc.scalar.activation(q[:], qp[:], AF.Identity, bias=bqs[:])
                qs.append(q)

            ep = epp.tile([K, L], f32, name="ep", tag="ep")
            nc.tensor.matmul(ep[:], k1s[:], qs[0][:], start=True, stop=False)
            nc.tensor.matmul(ep[:], k2ns[:], qs[1][:], start=False, stop=True)

            aabs = softp.tile([K, L], f32, name="aabs", tag="aabs")
            nc.scalar.activation(aabs[:], ep[:], AF.Abs)
            expe = softp.tile([K, L], bf16, name="expe", tag="expe")
            nc.scalar.activation(expe[:], aabs[:], AF.Exp)

            sp = spp.tile([1, L], f32, name="sp", tag="sp")
            nc.tensor.matmul(sp[:], oncs[:], expe[:], start=True, stop=True)
            rs = softp.tile([1, L], f32, name="rs", tag="rs")
            # 1/S at ~18 bits; S in [K, K*exp(~20)] so no edge cases
            nc.vector.reciprocal_approx_fast(rs[:], sp[:])
            rsb = softp.tile([1, L], bf16, name="rsb", tag="rsb")
            nc.scalar.copy(rsb[:], rs[:])
            rbp = spp.tile([K, L], f32, name="rbp", tag="rbp")
            nc.tensor.matmul(rbp[:], onrs[:], rsb[:], start=True, stop=True)
            attn = softp.tile([K, L], bf16, name="attn", tag="attn")
            nc.vector.tensor_mul(attn[:], expe[:], rbp[:])

            for s, (vs, orr) in enumerate(((v1s, o1r), (v2s, o2r))):
                ot = opool.tile([P, NCH * L], f32, name=f"ot{s}", tag=f"ot{s}")
                for j in range(NCH):
                    up = upp.tile([P, L], f32, name="up", tag="up")
                    nc.tensor.matmul(up[:], vs[:, j * P:(j + 1) * P],
                                     attn[:], start=True, stop=True)
                    # ot = (up * scale) + x in one DVE op
                    nc.vector.scalar_tensor_tensor(
                        ot[:, j * L:(j + 1) * L], up[:], scols[:],
                        xts[s][:, j * L:(j + 1) * L].bitcast(f32),
                        ALU.mult, ALU.add)
                    # stream-0 stores ride the SWDGE (gpsimd) queues,
                    # stream-1 the Activation HWDGE ring; the SP ring
                    # stays dedicated to loads
                    steng = nc.gpsimd if s == 0 else nc.scalar
                    steng.dma_start(orr[j][:, l0:l0 + L],
                                    ot[:, j * L:(j + 1) * L])

    nc.compile()
    return nc


def _get_nc():
    if "nc" not in _CACHE:
        try:
            import concourse  # noqa: F401
        except ImportError:
            import sys
            sys.path.insert(0, "/opt/trn_rl_repo")
        _CACHE["nc"] = _build()
    return _CACHE["nc"]


def _bf16_np():
    import ml_dtypes
    return ml_dtypes.bfloat16


def _make_in_maps(inputs):
    def f32(a):
        return np.ascontiguousarray(np.asarray(a, dtype=np.float32))

    bf = _bf16_np()

    def b16(a):
        return np.ascontiguousarray(np.asarray(a).astype(bf))

    x1 = f32(inputs["x1"]).reshape(N, C, HW)
    x2 = f32(inputs["x2"]).reshape(N, C, HW)
    y1 = np.asarray(inputs["y1"])
    y2 = np.asarray(inputs["y2"])
    shared = {
        "wq1t": f32(np.asarray(inputs["wq1"]).T),
        "wq2t": f32(np.asarray(inputs["wq2"]).T),
        "wk1t": b16(np.asarray(inputs["wk1"]).T),
        "wk2t": b16(np.asarray(inputs["wk2"]).T),
        "wv1t": b16(np.asarray(inputs["wv1"]).T),
        "wv2t": b16(np.asarray(inputs["wv2"]).T),
        "bq1": f32(inputs["bq1"]).reshape(D, 1),
        "bq2": f32(inputs["bq2"]).reshape(D, 1),
        "bk1": f32(inputs["bk1"]).reshape(D, 1),
        "bk2": f32(inputs["bk2"]).reshape(D, 1),
        "bv1": b16(np.asarray(inputs["bv1"]).reshape(1, C)),
        "bv2": b16(np.asarray(inputs["bv2"]).reshape(1, C)),
        "ones_r": np.ones((1, K), bf),
        "ones_c": np.ones((K, 1), bf),
        "scol": np.full((P, 1), np.asarray(inputs["scale"]).reshape(-1)[0],
                        dtype=np.float32),
    }
    in_maps = []
    for i in range(N):
        m = dict(shared)
        m["x1"] = x1[i]
        m["x2"] = x2[i]
        m["y1t"] = b16(y1[i].T)
        m["y2t"] = b16(y2[i].T)
        in_maps.append(m)
    return in_maps


def kernel(**inputs):
    nc = _get_nc()
    from concourse.bass_utils import run_bass_kernel_spmd

    in_maps = _make_in_maps(inputs)
    res = run_bass_kernel_spmd(nc, in_maps, list(range(N))).results
    out1 = np.stack([res[i]["o1"] for i in range(N)]).reshape(N, C, H, W)
    out2 = np.stack([res[i]["o2"] for i in range(N)]).reshape(N, C, H, W)
    return out1, out2



# revision 2
# speedup vs baseline: 1.0562x; 1.0562x over previous
"""Trainium2 Bass kernel for nn_CPAMDec_Mix (dual cross-attention, CPAM decoder).

Math (per batch element n):
    q_i = wq_i @ x_i + bq_i              # (D, HW)   1x1 conv query
    k_i = y_i @ wk_i.T + bk_i            # (K, D)    linear key
    v_i = y_i @ wv_i.T + bv_i            # (K, C)    linear value
    e   = | k_1 q_1 - k_2 q_2 |          # (K, HW)   (transposed layout)
    a   = softmax_K(e)
    out_i = scale * (v_i.T @ a) + x_i    # (C, HW)

Sharding: pure data parallel, one batch element per NeuronCore (N=8, 8 cores).

v6 design notes (trace-driven, see v4/v5):
  * bf16 HBM I/O, packed [128, NT*4*L] x/out layouts, 1 DMA per stream
    per tile, packed consts (3 DMAs).
  * 4/5-deep software pipeline; V-setup matmuls are EMITTED after step 1
    so the in-order PE reaches Q(0) without waiting for the wv DMA.
  * E in bf16 (fp8 q/k moved the scale=1 error to 7e-2: |logits| reach
    ~20, so relative q/k error becomes large absolute logit error).
  * softmax denominator: recip on DVE, bf16 copy on DVE (2x mode),
    partition-broadcast on GPSIMD (replaces the ones-matmul), attn
    multiply all-bf16 on DVE (2x mode).  GPSIMD runs ONLY pb: stores
    ride the SP ring (loads have drained by the time stores begin), so
    pb is never HOL-blocked behind a store's semaphore wait (v5 bug:
    12us attnmul stalls).
  * U matmuls processed in tile PAIRS so each V-chunk stationary serves
    two matmuls (weight switch costs ~115ns extra; reuse hits 216ns);
    drains grouped per chunk to bound PSUM-buffer rotation.
  * PSUM (8 banks): qp 2 | ep 1 | sp 1 | up 3 | setup 1.
"""

import numpy as np

N, C, H, W, K = 8, 512, 64, 64, 64
HW = H * W          # 4096
D = C // 4          # 128
L = 512             # pixel tile size
NT = HW // L        # 8 tiles
NCH = C // 128      # 4 contraction chunks
P = 128

# residual-drain engine per chunk j per stream:
# 'v' = DVE tensor_add, 'a' = Act cast with identity-matmul accumulation
RESID_ENG = {0: ['v', 'a', 'v', 'v'], 1: ['v', 'a', 'a', 'v']}

_SEG = {}
_off = 0
for _name, _w in (("y1", NCH * K), ("y2", NCH * K), ("wk1", NCH * D),
                  ("wk2", NCH * D), ("wq1", NCH * D), ("wq2", NCH * D),
                  ("ident", P), ("ones", K)):
    _SEG[_name] = (_off, _w)
    _off += _w
CPW = _off
WVW = 2 * C + 2 * NCH * C  # [bv1|bv2 on row 0] ++ wv1 chunks ++ wv2 chunks
FPW = 5  # f32 pack: bq1, bq2, bk1, bk2, scol

_CACHE = {}


def _build():
    from contextlib import ExitStack

    import concourse.tile as tile
    from concourse import bacc, mybir

    f32 = mybir.dt.float32
    bf16 = mybir.dt.bfloat16
    AF = mybir.ActivationFunctionType

    nc = bacc.Bacc("TRN2", target_bir_lowering=False, debug=False)

    def din(name, shape, dt=f32):
        return nc.dram_tensor(name, shape, dt, kind="ExternalInput").ap()

    def dout(name, shape, dt=bf16):
        return nc.dram_tensor(name, shape, dt, kind="ExternalOutput").ap()

    x1 = din("x1", [P, NT * NCH * L], bf16)
    x2 = din("x2", [P, NT * NCH * L], bf16)
    cpk = din("cpk", [P, CPW], bf16)
    fpk = din("fpk", [P, FPW], f32)
    wvp = din("wvp", [P, WVW], bf16)
    o1 = dout("o1", [P, NT * NCH * L])
    o2 = dout("o2", [P, NT * NCH * L])

    with tile.TileContext(nc) as tc, ExitStack() as ctx:
        cpool = ctx.enter_context(tc.tile_pool(name="const", bufs=1))

        # --- constants: packed DMAs; only x loads ride SP before wvp -------
        cps = cpool.tile([P, CPW], bf16, name="cps", tag="cps")
        nc.scalar.dma_start(cps[:], cpk[:])
        fps = cpool.tile([P, FPW], f32, name="fps", tag="fps")
        nc.scalar.dma_start(fps[:], fpk[:])
        wvs = cpool.tile([P, WVW], bf16, name="wvs", tag="wvs")

        def seg(name):
            o, w = _SEG[name]
            return cps[:, o:o + w]

        y1s, y2s = seg("y1"), seg("y2")
        wk1s, wk2s = seg("wk1"), seg("wk2")
        wq1s, wq2s = seg("wq1"), seg("wq2")
        idents = seg("ident")
        o_on, _ = _SEG["ones"]
        onrs = cps[0:1, o_on:o_on + K]  # [1,K] ones row
        oncs = cps[0:K, o_on:o_on + 1]  # [K,1] ones column
        bq1s = fps[0:D, 0:1]
        bq2s = fps[0:D, 1:2]
        bk1s = fps[0:D, 2:3]
        bk2s = fps[0:D, 3:4]
        scols = fps[0:K, 4:5]

        bk2n = cpool.tile([D, 1], f32, name="bk2n", tag="bk2n")
        nc.scalar.mul(bk2n[:], bk2s, -1.0)

        # key pack [D, 2K]: cols 0:K = k1, K:2K = -(k2+bk2)
        k12 = cpool.tile([D, 2 * K], bf16, name="k12", tag="k12")
        v1s = cpool.tile([K, C], bf16, name="v1s", tag="v1s")
        v2s = cpool.tile([K, C], bf16, name="v2s", tag="v2s")
        bvrow = cpool.tile([1, 2 * C], bf16, name="bvrow", tag="bvrow")

        # --- streaming pools ----------------------------------------------
        xpool = ctx.enter_context(tc.tile_pool(name="xpool", bufs=8))
        qsb = ctx.enter_context(tc.tile_pool(name="qsb", bufs=2))
        softp = ctx.enter_context(tc.tile_pool(name="softp", bufs=3))
        opool = ctx.enter_context(tc.tile_pool(name="opool", bufs=3))
        qpp = ctx.enter_context(tc.tile_pool(name="qpp", bufs=1, space="PSUM"))
        epp = ctx.enter_context(tc.tile_pool(name="epp", bufs=1, space="PSUM"))
        spp = ctx.enter_context(tc.tile_pool(name="spp", bufs=1, space="PSUM"))
        upp = ctx.enter_context(tc.tile_pool(name="upp", bufs=3, space="PSUM"))
        stpp = ctx.enter_context(tc.tile_pool(name="stpp", bufs=1,
                                              space="PSUM"))
        setp = stpp.tile([P, L], f32, name="setp", tag="setp")

        # --- K setup (needed by E(0) at step 1): uses setp bank ------------
        for (wks, ys, cofs, bias, sc) in (
                (wk1s, y1s, 0, bk1s, 1.0),
                (wk2s, y2s, K, bk2n[:], -1.0)):
            kp = setp[0:D, cofs:cofs + K]
            for j in range(NCH):
                nc.tensor.matmul(
                    kp, wks[:, j * D:(j + 1) * D],
                    ys[:, j * K:(j + 1) * K],
                    start=(j == 0), stop=(j == NCH - 1))
            nc.scalar.activation(k12[:, cofs:cofs + K], kp, AF.Identity,
                                 bias=bias, scale=sc)

        def setup_v():
            # bv1|bv2 live on partition row 0 of the wv pack's first 2C cols
            nc.scalar.copy(bvrow[:], wvs[0:1, 0:2 * C])
            for (ys, wvss, bvofs, vs) in (
                    (y1s, wvs[:, 2 * C:2 * C + NCH * C], 0, v1s),
                    (y2s, wvs[:, 2 * C + NCH * C:WVW], C, v2s)):
                vp = setp[0:K, :]
                for j in range(NCH):
                    nc.tensor.matmul(
                        vp, ys[:, j * K:(j + 1) * K],
                        wvss[:, j * C:(j + 1) * C],
                        start=(j == 0), stop=False)
                nc.tensor.matmul(vp, onrs, bvrow[:, bvofs:bvofs + C],
                                 start=False, stop=True)
                # fold runtime scale into V so the residual is a plain add
                nc.scalar.activation(vs[:], vp, AF.Identity, scale=scols)

        xts = {}    # t -> {s: tile}
        qs = {}     # t -> q12 tile [D, 2L] fp8
        expes = {}  # t -> expe tile
        rsbs = {}   # t -> rsb tile
        attns = {}  # t -> attn tile

        def stage_load(t):
            c0 = t * NCH * L
            xts[t] = {}
            for s, xr in ((0, x1), (1, x2)):
                xt = xpool.tile([P, NCH * L], bf16, name=f"x{s}", tag=f"x{s}")
                nc.sync.dma_start(xt[:], xr[:, c0:c0 + NCH * L])
                xts[t][s] = xt

        def stage_q(t):
            q12 = qsb.tile([D, 2 * L], bf16, name="q12", tag="q12")
            for s, (wqss, bqs) in enumerate(((wq1s, bq1s), (wq2s, bq2s))):
                qp = qpp.tile([D, L], f32, name=f"q{s}p", tag=f"q{s}p")
                for j in range(NCH):
                    nc.tensor.matmul(
                        qp[:],
                        wqss[:, j * D:(j + 1) * D],
                        xts[t][s][:, j * L:(j + 1) * L],
                        start=(j == 0), stop=(j == NCH - 1))
                nc.scalar.activation(q12[:, s * L:(s + 1) * L], qp[:],
                                     AF.Identity, bias=bqs)
            qs[t] = q12

        def stage_e(t):
            ep = epp.tile([K, L], f32, name="ep", tag="ep")
            nc.tensor.matmul(ep[:], k12[:, 0:K], qs[t][:, 0:L],
                             start=True, stop=False)
            nc.tensor.matmul(ep[:], k12[:, K:2 * K], qs[t][:, L:2 * L],
                             start=False, stop=True)
            del qs[t]
            aabs = softp.tile([K, L], f32, name="aabs", tag="aabs")
            nc.scalar.activation(aabs[:], ep[:], AF.Abs)
            expe = softp.tile([K, L], bf16, name="expe", tag="expe")
            nc.scalar.activation(expe[:], aabs[:], AF.Exp)
            expes[t] = expe

        def stage_sum(t):
            sp = spp.tile([1, L], f32, name="sp", tag="sp")
            nc.tensor.matmul(sp[:], oncs, expes[t][:], start=True, stop=True)
            rs = softp.tile([1, L], f32, name="rs", tag="rs")
            # 1/S at ~18 bits; S in [K, K*exp(~20)] so no edge cases
            nc.vector.reciprocal_approx_fast(rs[:], sp[:])
            rsb = softp.tile([1, L], bf16, name="rsb", tag="rsb")
            nc.vector.tensor_copy(rsb[:], rs[:])
            rsbs[t] = rsb

        def stage_bcast(t):
            rbp = softp.tile([K, L], bf16, name="rbp", tag="rbp")
            nc.gpsimd.partition_broadcast(rbp[:], rsbs[t][:])
            del rsbs[t]
            attn = softp.tile([K, L], bf16, name="attn", tag="attn")
            nc.vector.tensor_mul(attn[:], expes[t][:], rbp[:])
            del expes[t]
            attns[t] = attn

        def stage_out_pair(t0, t1):
            pair = [t0, t1] if t1 is not None else [t0]
            atts = {t: attns.pop(t) for t in pair}
            for s, (vs, odr) in enumerate(((v1s, o1), (v2s, o2))):
                ots = {}
                for t in pair:
                    ots[t] = opool.tile([P, NCH * L], bf16, name=f"ot{s}",
                                        tag=f"ot{s}")
                # per chunk: V matmuls for both tiles share the stationary;
                # drains grouped right after to bound up-buffer rotation
                for j in range(NCH):
                    acc = RESID_ENG[s][j] == 'a'
                    ups = {}
                    for t in pair:
                        up = upp.tile([P, L], f32, name="up", tag="up")
                        nc.tensor.matmul(up[:], vs[:, j * P:(j + 1) * P],
                                         atts[t][:], start=True, stop=not acc)
                        ups[t] = up
                    if acc:
                        for t in pair:
                            nc.tensor.matmul(
                                ups[t][:], idents,
                                xts[t][s][:, j * L:(j + 1) * L],
                                start=False, stop=True)
                    for t in pair:
                        if acc:
                            nc.scalar.activation(
                                ots[t][:, j * L:(j + 1) * L],
                                ups[t][:], AF.Identity)
                        else:
                            nc.vector.tensor_add(
                                ots[t][:, j * L:(j + 1) * L], ups[t][:],
                                xts[t][s][:, j * L:(j + 1) * L])
                for t in pair:
                    c0 = t * NCH * L
                    nc.sync.dma_start(odr[:, c0:c0 + NCH * L], ots[t][:])
            for t in pair:
                del xts[t]

        PRE = 3  # x loads issued this many steps ahead of stage_q
        for step in range(NT + 4):
            if step == 0:
                for tt in range(min(PRE, NT)):
                    stage_load(tt)
                # wv pack load rides SP *after* the prologue x loads
                nc.sync.dma_start(wvs[:], wvp[:])
            if step + PRE < NT:
                stage_load(step + PRE)
            if step < NT:
                stage_q(step)
            if 0 <= step - 1 < NT:
                stage_e(step - 1)
            if step == 1:
                # V setup here: PE reaches it after Q(0..1)/E(0), by which
                # time the wv DMA has landed; consumers start at step 5
                setup_v()
            if 0 <= step - 2 < NT:
                stage_sum(step - 2)
            if 0 <= step - 3 < NT:
                stage_bcast(step - 3)
            if step >= 4 and (step - 4) % 2 == 0 and step - 4 < NT:
                # pair (t, t+1) fires the same step bcast(t+1) runs: tile
                # t's U matmuls cover the attnmul(t+1) latency
                stage_out_pair(step - 4, step - 3 if step - 3 < NT else None)

    nc.compile()
    return nc


def _get_nc():
    if "nc" not in _CACHE:
        try:
            import concourse  # noqa: F401
        except ImportError:
            import sys
            sys.path.insert(0, "/opt/trn_rl_repo")
        _CACHE["nc"] = _build()
    return _CACHE["nc"]


def _bf16_np():
    import ml_dtypes
    return ml_dtypes.bfloat16


def _pack_x(x):
    """[C, HW] f32 -> [128, NT*4*512] bf16 packed pixel-tile-major."""
    bf = _bf16_np()
    v = x.reshape(NCH, P, NT, L).transpose(1, 2, 0, 3)
    return np.ascontiguousarray(v.astype(bf)).reshape(P, NT * NCH * L)


def _unpack_o(o):
    """[128, NT*4*512] bf16 -> [C, HW] f32."""
    v = o.reshape(P, NT, NCH, L).transpose(2, 0, 1, 3)
    return np.ascontiguousarray(v, dtype=np.float32).reshape(C, HW)


def _chunked(a):
    """[C, W] -> [128, NCH*W] with chunks side by side."""
    cw = a.shape[1]
    return np.ascontiguousarray(
        a.reshape(NCH, P, cw).transpose(1, 0, 2)).reshape(P, NCH * cw)


def _make_in_maps(inputs):
    def f32(a):
        return np.ascontiguousarray(np.asarray(a, dtype=np.float32))

    bf = _bf16_np()

    def b16(a):
        return np.ascontiguousarray(np.asarray(a).astype(bf))

    x1 = np.asarray(inputs["x1"], dtype=np.float32).reshape(N, C, HW)
    x2 = np.asarray(inputs["x2"], dtype=np.float32).reshape(N, C, HW)
    y1 = np.asarray(inputs["y1"])
    y2 = np.asarray(inputs["y2"])

    scale = float(np.asarray(inputs["scale"]).reshape(-1)[0])

    fpk = np.zeros((P, FPW), np.float32)
    fpk[0:D, 0] = f32(inputs["bq1"]).reshape(-1)
    fpk[0:D, 1] = f32(inputs["bq2"]).reshape(-1)
    fpk[0:D, 2] = f32(inputs["bk1"]).reshape(-1)
    fpk[0:D, 3] = f32(inputs["bk2"]).reshape(-1)
    fpk[0:K, 4] = scale

    wv1t = b16(np.asarray(inputs["wv1"]).T)
    wv2t = b16(np.asarray(inputs["wv2"]).T)
    wvp = np.zeros((P, WVW), bf)
    wvp[0, 0:C] = b16(np.asarray(inputs["bv1"]).reshape(-1))
    wvp[0, C:2 * C] = b16(np.asarray(inputs["bv2"]).reshape(-1))
    wvp[:, 2 * C:2 * C + NCH * C] = _chunked(wv1t)
    wvp[:, 2 * C + NCH * C:] = _chunked(wv2t)

    cpk = np.zeros((P, CPW), bf)

    def put(name, arr):
        o, w = _SEG[name]
        cpk[:, o:o + w] = arr

    put("wk1", _chunked(b16(np.asarray(inputs["wk1"]).T)))
    put("wk2", _chunked(b16(np.asarray(inputs["wk2"]).T)))
    put("wq1", _chunked(b16(np.asarray(inputs["wq1"]).T)))
    put("wq2", _chunked(b16(np.asarray(inputs["wq2"]).T)))
    put("ident", np.eye(P, dtype=np.float32).astype(bf))
    o_on, w_on = _SEG["ones"]
    cpk[:, o_on:o_on + w_on] = 1.0

    in_maps = []
    for i in range(N):
        cp = cpk.copy()
        o, w = _SEG["y1"]
        cp[:, o:o + w] = _chunked(b16(y1[i].T))
        o, w = _SEG["y2"]
        cp[:, o:o + w] = _chunked(b16(y2[i].T))
        in_maps.append({
            "cpk": cp,
            "fpk": fpk,
            "wvp": wvp,
            "x1": _pack_x(x1[i]),
            "x2": _pack_x(x2[i]),
        })
    return in_maps


def kernel(**inputs):
    nc = _get_nc()
    from concourse.bass_utils import run_bass_kernel_spmd

    in_maps = _make_in_maps(inputs)
    res = run_bass_kernel_spmd(nc, in_maps, list(range(N))).results
    out1 = np.stack([_unpack_o(res[i]["o1"]) for i in range(N)])
    out2 = np.stack([_unpack_o(res[i]["o2"]) for i in range(N)])
    return out1.reshape(N, C, H, W), out2.reshape(N, C, H, W)
